# revision 8
# baseline (speedup 1.0000x reference)
"""Trainium2 Bass kernel for DPPDynamicEmbedding (retrieval_knn).

Reference computation (per batch b, N=4096 points in [0,1]^2):
  placed    = (~action_mask) & ~(keepout | probe)                  [N] bool
  d2[i,j]   = |x_i|^2 + |x_j|^2 - 2 x_i.x_j                        [N,N]
  density_i = |{j : placed_j and d2[i,j] < R^2}| / 20              [N]
  proj      = [placed, density] @ W                                [N, 384]
  out       = split(proj, 3) -> (glimpse_key, glimpse_val, logit_key)

Strategy: data-parallel, 2 batches per core on 8 cores.  Per i-block of
128 points (i = p*C + ib, p = SBUF partition, C = N/128):

  - PE computes psum[i, j] = -2 x_i.x_j + masked_sq_j with K=8 fp16
    matmuls.  fp16 hi/lo splitting (x = xh + xl, sq = sqh + sql) keeps
    d2 accurate to ~1e-6 while running single-pass (fp32 matmuls are
    2-pass on TRN2 and much slower).  The j-domain is compacted on the
    host to only placed points, padded to J; pad entries get sq = 32768
    so they can never be inside the radius.
  - K=8 fits a 32-row PE group, so the per-block matmuls cycle through
    array row groups 0/32/64/96 via tile_position; weights and moving
    operands are replicated at partition offsets 0/32/64/96 so the four
    chunk matmuls (and the rotating projection matmul) overlap in the
    array instead of serializing on the ~(219+N)cyc isolated-MM latency.
  - The comparison d2 < R^2  <=>  psum < thresh_i (thresh = R^2 - sq_i)
    is one fused compare+count op per engine per i-block, reading a
    2-bank PSUM tile:
      ACT: Sign(thresh - psum), accum_out S  => count = (S + Wa)/2
      DVE: tensor_scalar is_lt, accum_out C  => count = C
  - Counts (fp32, converted to exact fp16 integers) take a tiny DRAM
    round-trip to become rows [1, N] (p-major flatten == i order) of
    feat^T, written into all four row-group replicas.
  - Projection is one K=8 fp16 matmul per 128 points: lhsT rows =
    [placed, placed, S, S, C, C, 1, 1], rhs rows = hi/lo splits of
    [W0, W1/40, W1/20, (Wa/40) W1] -- count normalization and the
    Sign->count affine fix ride the matmul; the hi/lo W split keeps
    fp32-level accuracy.  PSUM -> SBUF copy (ACT/DVE split; DMA cannot
    read PSUM) then one DMA per two i-blocks into a [N, 384] output.
"""

import numpy as np

import concourse.bass as bass
import concourse.mybir as mybir
import concourse.tile as tile
from concourse import bacc, bass_utils

R2 = 0.16
SCALE = 20.0
BIG = 32768.0          # pad sentinel; must be fp16-exact and >> R2
N_CORES = 8

F32 = mybir.dt.float32
F16 = mybir.dt.float16


def _split16(v):
    """Split fp32 array into (hi, lo) fp16 pair with v ~= hi + lo."""
    hi = v.astype(np.float16)
    lo = (v - hi.astype(np.float32)).astype(np.float16)
    return hi, lo


def _wa_for(J):
    """ACT-side j-width (DVE gets J - wa)."""
    return int(max(512, min(1024, round(J * 0.52 / 64) * 64)))


def _subchunks(w, lim=512):
    out = []
    off = 0
    while off < w:
        out.append((off, min(lim, w - off)))
        off += lim
    return out


def build_program(N, BPC, J, wa, copy_split=4):
    """N points, BPC batches per core, J padded j-count, wa = ACT width."""
    C = N // 128
    NB = N // 128
    wd = J - wa
    # chunk list: (psum kind, psum offset, rhs offset, width, row group)
    chunks = []
    g = 0
    for off, w in _subchunks(wa):
        chunks.append(("a", off, off, w, 32 * (g % 4)))
        g += 1
    for off, w in _subchunks(wd):
        chunks.append(("d", off, wa + off, w, 32 * (g % 4)))
        g += 1

    nc = bacc.Bacc("TRN2", target_bir_lowering=False, debug=False,
                   num_devices=N_CORES)

    xi_d = nc.dram_tensor("xi", [BPC, 128, N], F16, kind="ExternalInput")
    rhs3_d = nc.dram_tensor("rhs3", [BPC, 128, J], F16, kind="ExternalInput")
    th_d = nc.dram_tensor("thresh", [BPC, 128, C], F32, kind="ExternalInput")
    ft_d = nc.dram_tensor("featT", [BPC, 128, N], F16, kind="ExternalInput")
    rw_d = nc.dram_tensor("rhsW", [128, 384], F16, kind="ExternalInput")
    pj_d = nc.dram_tensor("proj", [BPC, N, 384], F32, kind="ExternalOutput")

    with tile.TileContext(nc) as tc:
        with (
            tc.tile_pool(name="const", bufs=BPC) as cpool,
            tc.tile_pool(name="accp", bufs=BPC) as accp,
            tc.tile_pool(name="pa", bufs=2, space="PSUM") as pap,
            tc.tile_pool(name="pdv", bufs=2, space="PSUM") as pdp,
            tc.tile_pool(name="scr_a", bufs=2) as scra,
            tc.tile_pool(name="scr_d", bufs=2) as scrd,
            tc.tile_pool(name="outsb", bufs=2) as outp,
            tc.tile_pool(name="dram", bufs=BPC, space="DRAM") as dram,
            tc.tile_pool(name="w", bufs=1) as wpool,
        ):
            rhsW = wpool.tile([128, 384], F16)
            nc.sync.dma_start(rhsW[:], rw_d.ap())

            xi, rhs3, th, ft, acc_a, acc_d, sd_a, sd_d = \
                [], [], [], [], [], [], [], []
            for b in range(BPC):
                t = cpool.tile([128, N], F16, tag="xi", name=f"xi{b}")
                nc.sync.dma_start(t[:], xi_d.ap()[b])
                xi.append(t)
                t = cpool.tile([128, J], F16, tag="rhs3", name=f"rhs3{b}")
                nc.sync.dma_start(t[:], rhs3_d.ap()[b])
                rhs3.append(t)
                t = cpool.tile([128, C], F32, tag="th", name=f"th{b}")
                nc.sync.dma_start(t[:], th_d.ap()[b])
                th.append(t)
                t = cpool.tile([128, N], F16, tag="ft", name=f"ft{b}")
                nc.sync.dma_start(t[:], ft_d.ap()[b])
                ft.append(t)
                acc_a.append(accp.tile([128, C], F32, tag="aa", name=f"aa{b}"))
                acc_d.append(accp.tile([128, C], F32, tag="ad", name=f"ad{b}"))
                sd_a.append(dram.tile([128, C], F16, tag="sa", name=f"sda{b}"))
                sd_d.append(dram.tile([128, C], F16, tag="sd", name=f"sdd{b}"))

            # ---- fused phases: counts(b) interleaved with proj(b-1) ----
            def counts_block(b, ib):
                isl = slice(ib * 128, (ib + 1) * 128)
                pa = pap.tile([128, 1024], F32, tag="pa", name=f"pa_{b}_{ib}")
                pd = pdp.tile([128, 1024], F32, tag="pd", name=f"pd_{b}_{ib}")
                for kind, po_, ro, w, grp in chunks:
                    dst = pa if kind == "a" else pd
                    nc.tensor.matmul(
                        dst[:, po_:po_ + w],
                        xi[b][grp:grp + 8, isl],
                        rhs3[b][grp:grp + 8, ro:ro + w],
                        start=True, stop=True,
                        tile_position=(grp, 0))
                sa = scra.tile([128, 1024], F16, tag="sa", name=f"sa_{b}_{ib}")
                nc.scalar.activation(
                    sa[:, :wa], pa[:, :wa],
                    mybir.ActivationFunctionType.Sign,
                    bias=th[b][:, ib:ib + 1], scale=-1.0,
                    accum_out=acc_a[b][:, ib:ib + 1])
                sd = scrd.tile([128, 1024], F16, tag="sd", name=f"sd_{b}_{ib}")
                nc.vector.tensor_scalar(
                    sd[:, :wd], pd[:, :wd],
                    th[b][:, ib:ib + 1], None,
                    op0=mybir.AluOpType.is_lt,
                    op1=mybir.AluOpType.add,
                    accum_out=acc_d[b][:, ib:ib + 1])

            def roundtrip(b):
                # counts: fp32 -> exact fp16 ints -> DRAM -> featT rows
                a16 = accp.tile([128, C], F16, tag="a16", name=f"a16_{b}")
                nc.scalar.copy(a16[:], acc_a[b][:])
                d16 = accp.tile([128, C], F16, tag="d16", name=f"d16_{b}")
                nc.scalar.copy(d16[:], acc_d[b][:])
                nc.sync.dma_start(sd_a[b][:, :], a16[:])
                nc.sync.dma_start(sd_d[b][:, :], d16[:])
                row_a = sd_a[b][:, :].rearrange("p c -> (p c)") \
                    .rearrange("(a n) -> a n", a=1)
                row_d = sd_d[b][:, :].rearrange("p c -> (p c)") \
                    .rearrange("(a n) -> a n", a=1)
                # featT rows per group: [pl, pl, S, S, C, C, 1, 1]
                for grp in (0, 32, 64, 96):
                    nc.sync.dma_start(ft[b][grp + 2:grp + 3, :], row_a)
                    nc.sync.dma_start(ft[b][grp + 3:grp + 4, :], row_a)
                    nc.sync.dma_start(ft[b][grp + 4:grp + 5, :], row_d)
                    nc.sync.dma_start(ft[b][grp + 5:grp + 6, :], row_d)

            def proj_pair(b, ib2):
                osb = outp.tile([128, 768], F32, tag="osb",
                                name=f"osb_{b}_{ib2}")
                pool = pap if (ib2 // 2) % 2 == 0 else pdp
                tag = "pa" if pool is pap else "pd"
                po = pool.tile([128, 1024], F32, tag=tag,
                               name=f"po_{b}_{ib2}")
                for s_ in range(2):
                    ib = ib2 + s_
                    grp = 32 * (ib % 4)
                    nc.tensor.matmul(
                        po[:, s_ * 512:s_ * 512 + 384],
                        ft[b][grp:grp + 8, ib * 128:(ib + 1) * 128],
                        rhsW[grp:grp + 8, :], start=True, stop=True,
                        tile_position=(grp, 0))
                pov = po[:].rearrange("p (s k) -> p s k", s=2)[:, :, :384]
                if (ib2 // 2) % 2 == 0:
                    nc.vector.tensor_copy(
                        osb[:].rearrange("p (s k) -> p s k", s=2), pov)
                else:
                    nc.scalar.copy(
                        osb[:].rearrange("p (s k) -> p s k", s=2), pov)
                dst = pj_d.ap()[b, ib2 * 128:(ib2 + 2) * 128, :] \
                    .rearrange("(s p) k -> p s k", p=128)
                nc.sync.dma_start(
                    dst, osb[:].rearrange("p (s k) -> p s k", s=2))

            for b in range(BPC):
                for ib in range(NB):
                    counts_block(b, ib)
                    if b > 0 and ib % 2 == 1:
                        proj_pair(b - 1, ib - 1)
                roundtrip(b)
            for ib2 in range(0, NB, 2):
                proj_pair(BPC - 1, ib2)
    nc.compile()
    return nc


def prep_core_inputs(action_mask, keepout, probe, locs, W, J, wa):
    """Host-side prep for one core's batches. Returns in_map dict."""
    BPC, N, _ = locs.shape
    C = N // 128

    placed = (~action_mask) & ~(keepout | probe)          # [BPC, N] bool
    placed_f = placed.astype(np.float32)
    x = locs.astype(np.float32)
    sq = (x ** 2).sum(-1)                                 # [BPC, N]
    thresh = (R2 - sq).astype(np.float32)

    # p-major i-block layout: lhsT col m of block ib <- i = m*C + ib
    m = np.arange(128)
    src = (m[None, :] * C + np.arange(C)[:, None]).reshape(-1)  # pos ib*128+m

    xi = np.zeros((BPC, 128, N), np.float16)
    rhs3 = np.zeros((BPC, 128, J), np.float16)
    featT = np.zeros((BPC, 128, N), np.float16)
    th_pm = np.zeros((BPC, 128, C), np.float32)

    for b in range(BPC):
        x0h, x0l = _split16(x[b, :, 0])
        x1h, x1l = _split16(x[b, :, 1])

        idx = np.nonzero(placed[b])[0]
        np_ = len(idx)
        assert np_ <= J, f"placed count {np_} exceeds J={J}"
        j0h, j0l = _split16(-2.0 * x[b, idx, 0])
        j1h, j1l = _split16(-2.0 * x[b, idx, 1])
        sqh, sql = _split16(sq[b, idx])

        for grp in (0, 32, 64, 96):
            # lhsT rows: [xh0, xh0, xl0, xh1, xh1, xl1, 1, 1]
            xi[b, grp + 0, :] = x0h[src]
            xi[b, grp + 1, :] = x0h[src]
            xi[b, grp + 2, :] = x0l[src]
            xi[b, grp + 3, :] = x1h[src]
            xi[b, grp + 4, :] = x1h[src]
            xi[b, grp + 5, :] = x1l[src]
            xi[b, grp + 6, :] = 1.0
            xi[b, grp + 7, :] = 1.0
            # rhs rows pair to give xh*(-2xh) + xh*(-2xl) + xl*(-2xh)
            # per coord, plus 1*sqh + 1*sql (pads: sq = BIG).
            rhs3[b, grp + 6, :] = BIG
            rhs3[b, grp + 0, :np_] = j0h
            rhs3[b, grp + 1, :np_] = j0l
            rhs3[b, grp + 2, :np_] = j0h
            rhs3[b, grp + 3, :np_] = j1h
            rhs3[b, grp + 4, :np_] = j1l
            rhs3[b, grp + 5, :np_] = j1h
            rhs3[b, grp + 6, :np_] = sqh
            rhs3[b, grp + 7, :np_] = sql
            # featT rows: [pl, pl, S, S, C, C, 1, 1]; S/C filled on device
            featT[b, grp + 0, :] = placed_f[b]
            featT[b, grp + 1, :] = placed_f[b]
            featT[b, grp + 6, :] = 1.0
            featT[b, grp + 7, :] = 1.0

        th_pm[b] = thresh[b].reshape(128, C)

    W = W.astype(np.float32)
    rhsW = np.zeros((128, 384), np.float16)
    rows = [W[0],                         # placed
            W[1] / (2.0 * SCALE),         # S (ACT sign-sum)
            W[1] / SCALE,                 # C (DVE count)
            (wa / (2.0 * SCALE)) * W[1]]  # ones (Sign affine fix)
    for grp in (0, 32, 64, 96):
        for r, v in enumerate(rows):
            h, lo = _split16(v)
            rhsW[grp + 2 * r] = h
            rhsW[grp + 2 * r + 1] = lo

    return {"xi": xi, "rhs3": rhs3, "thresh": th_pm, "featT": featT,
            "rhsW": rhsW}


_PROGRAM_CACHE = {}


def kernel(action_mask, keepout, probe, locs, W, _trace=False, _tmpdir=None):
    action_mask = np.asarray(action_mask)
    keepout = np.asarray(keepout)
    probe = np.asarray(probe)
    locs = np.asarray(locs, dtype=np.float32)
    W = np.asarray(W, dtype=np.float32)

    B, N = action_mask.shape
    BPC = B // N_CORES

    placed = (~action_mask) & ~(keepout | probe)
    max_placed = int(placed.sum(1).max())
    J = max(1536, ((max_placed + 63) // 64) * 64)
    wa = _wa_for(J)

    key = (N, BPC, J, wa)
    if key not in _PROGRAM_CACHE:
        _PROGRAM_CACHE[key] = build_program(N, BPC, J, wa)
    nc = _PROGRAM_CACHE[key]

    in_maps = []
    for c in range(N_CORES):
        s = slice(c * BPC, (c + 1) * BPC)
        in_maps.append(prep_core_inputs(
            action_mask[s], keepout[s], probe[s], locs[s], W, J, wa))

    res = bass_utils.run_bass_kernel_spmd(
        nc, in_maps, core_ids=list(range(N_CORES)),
        trace=_trace, tmpdir=_tmpdir)

    proj = np.concatenate([res.results[c]["proj"] for c in range(N_CORES)], 0)
    out = (np.ascontiguousarray(proj[:, :, :128]),
           np.ascontiguousarray(proj[:, :, 128:256]),
           np.ascontiguousarray(proj[:, :, 256:384]))
    if _trace:
        return out, res
    return out


# revision 10
# speedup vs baseline: 1.0618x; 1.0618x over previous
"""Trainium2 Bass kernel for DPPDynamicEmbedding (retrieval_knn).

Reference computation (per batch b, N=4096 points in [0,1]^2):
  placed    = (~action_mask) & ~(keepout | probe)                  [N] bool
  d2[i,j]   = |x_i|^2 + |x_j|^2 - 2 x_i.x_j                        [N,N]
  density_i = |{j : placed_j and d2[i,j] < R^2}| / 20              [N]
  proj      = [placed, density] @ W                                [N, 384]
  out       = split(proj, 3) -> (glimpse_key, glimpse_val, logit_key)

Strategy: data-parallel, 2 batches per core on 8 cores.  Per i-block of
128 points (i = p*C + ib, p = SBUF partition, C = N/128):

  - PE computes psum[i, j] = -2 x_i.x_j + masked_sq_j with K=8 fp16
    matmuls.  fp16 hi/lo splitting (x = xh + xl, sq = sqh + sql) keeps
    d2 accurate to ~1e-6 while running single-pass (fp32 matmuls are
    2-pass on TRN2 and much slower).  The j-domain is compacted on the
    host to only placed points, padded to J; pad entries get sq = 32768
    so they can never be inside the radius.
  - K=8 fits a 32-row PE group, so the per-block matmuls cycle through
    array row groups 0/32/64/96 via tile_position; weights and moving
    operands are replicated at partition offsets 0/32/64/96 so the four
    chunk matmuls (and the rotating projection matmul) overlap in the
    array instead of serializing on the ~(219+N)cyc isolated-MM latency.
  - The comparison d2 < R^2  <=>  psum < thresh_i (thresh = R^2 - sq_i)
    is one fused compare+count op per engine per i-block, reading a
    2-bank PSUM tile:
      ACT: Sign(thresh - psum), accum_out S  => count = (S + Wa)/2
      DVE: tensor_scalar is_lt, accum_out C  => count = C
  - Counts (fp32, converted to exact fp16 integers) take a tiny DRAM
    round-trip to become rows [1, N] (p-major flatten == i order) of
    feat^T, written into all four row-group replicas.
  - Projection is one K=8 fp16 matmul per 128 points: lhsT rows =
    [placed, placed, S, S, C, C, 1, 1], rhs rows = hi/lo splits of
    [W0, W1/40, W1/20, (Wa/40) W1] -- count normalization and the
    Sign->count affine fix ride the matmul; the hi/lo W split keeps
    fp32-level accuracy.  PSUM -> SBUF copy (ACT/DVE split; DMA cannot
    read PSUM) then one DMA per two i-blocks into a [N, 384] output.
"""

import numpy as np

import concourse.bass as bass
import concourse.mybir as mybir
import concourse.tile as tile
from concourse import bacc, bass_utils

R2 = 0.16
SCALE = 20.0
BIG = 32768.0          # pad sentinel; must be fp16-exact and >> R2
N_CORES = 8

F32 = mybir.dt.float32
F16 = mybir.dt.float16


def _split16(v):
    """Split fp32 array into (hi, lo) fp16 pair with v ~= hi + lo."""
    hi = v.astype(np.float16)
    lo = (v - hi.astype(np.float32)).astype(np.float16)
    return hi, lo


def _wa_for(J):
    """ACT-side j-width (DVE gets J - wa)."""
    return int(max(512, min(1024, round(J * 0.52 / 64) * 64)))


def _subchunks(w, lim=512):
    out = []
    off = 0
    while off < w:
        out.append((off, min(lim, w - off)))
        off += lim
    return out


def build_program(N, BPC, J, wa, copy_split=4):
    """N points, BPC batches per core, J padded j-count, wa = ACT width."""
    C = N // 128
    NB = N // 128
    wd = J - wa
    # chunk list: (psum kind, psum offset, rhs offset, width, row group)
    chunks = []
    g = 0
    for off, w in _subchunks(wa):
        chunks.append(("a", off, off, w, 32 * (g % 4)))
        g += 1
    for off, w in _subchunks(wd):
        chunks.append(("d", off, wa + off, w, 32 * (g % 4)))
        g += 1

    nc = bacc.Bacc("TRN2", target_bir_lowering=False, debug=False,
                   num_devices=N_CORES)

    xi_d = nc.dram_tensor("xi", [BPC, 8, N], F16, kind="ExternalInput")
    rhs3_d = nc.dram_tensor("rhs3", [BPC, 8, J], F16, kind="ExternalInput")
    th_d = nc.dram_tensor("thresh", [BPC, 128, C], F32, kind="ExternalInput")
    ft_d = nc.dram_tensor("featT", [BPC, 8, N], F16, kind="ExternalInput")
    rw_d = nc.dram_tensor("rhsW", [8, 384], F16, kind="ExternalInput")
    pj_d = nc.dram_tensor("proj", [BPC, N, 384], F32, kind="ExternalOutput")

    with tile.TileContext(nc) as tc:
        with (
            tc.tile_pool(name="const", bufs=BPC) as cpool,
            tc.tile_pool(name="accp", bufs=BPC) as accp,
            tc.tile_pool(name="pa", bufs=2, space="PSUM") as pap,
            tc.tile_pool(name="pdv", bufs=2, space="PSUM") as pdp,
            tc.tile_pool(name="scr_a", bufs=2) as scra,
            tc.tile_pool(name="scr_d", bufs=2) as scrd,
            tc.tile_pool(name="outsb", bufs=4) as outp,
            tc.tile_pool(name="dram", bufs=BPC, space="DRAM") as dram,
            tc.tile_pool(name="w", bufs=1) as wpool,
        ):
            def load_repl(pool, tag, name, dram_ap, rows, cols, eng):
                t = pool.tile([128, cols], F16, tag=tag, name=name)
                nc.sync.dma_start(t[:rows, :], dram_ap)
                for grp in (32, 64, 96):
                    eng.dma_start(t[grp:grp + rows, :], t[:rows, :])
                return t

            rhsW = load_repl(wpool, "rw", "rhsW", rw_d.ap(), 8, 384,
                             nc.gpsimd)

            xi, rhs3, th, ft, acc_a, acc_d, sd_a, sd_d = \
                [], [], [], [], [], [], [], []
            for b in range(BPC):
                xi.append(load_repl(cpool, "xi", f"xi{b}", xi_d.ap()[b],
                                    8, N, nc.gpsimd))
                rhs3.append(load_repl(cpool, "rhs3", f"rhs3{b}",
                                      rhs3_d.ap()[b], 8, J, nc.gpsimd))
                t = cpool.tile([128, C], F32, tag="th", name=f"th{b}")
                nc.sync.dma_start(t[:], th_d.ap()[b])
                th.append(t)
                ft.append(load_repl(cpool, "ft", f"ft{b}", ft_d.ap()[b],
                                    8, N, nc.gpsimd))
                acc_a.append(accp.tile([128, C], F32, tag="aa", name=f"aa{b}"))
                acc_d.append(accp.tile([128, C], F32, tag="ad", name=f"ad{b}"))
                sd_a.append(dram.tile([128, C], F16, tag="sa", name=f"sda{b}"))
                sd_d.append(dram.tile([128, C], F16, tag="sd", name=f"sdd{b}"))

            # ---- fused phases: counts(b) interleaved with proj(b-1) ----
            def counts_block(b, ib):
                isl = slice(ib * 128, (ib + 1) * 128)
                pa = pap.tile([128, 1024], F32, tag="pa", name=f"pa_{b}_{ib}")
                pd = pdp.tile([128, 1024], F32, tag="pd", name=f"pd_{b}_{ib}")
                for kind, po_, ro, w, grp in chunks:
                    dst = pa if kind == "a" else pd
                    nc.tensor.matmul(
                        dst[:, po_:po_ + w],
                        xi[b][grp:grp + 8, isl],
                        rhs3[b][grp:grp + 8, ro:ro + w],
                        start=True, stop=True,
                        tile_position=(grp, 0))
                sa = scra.tile([128, 1024], F16, tag="sa", name=f"sa_{b}_{ib}")
                nc.scalar.activation(
                    sa[:, :wa], pa[:, :wa],
                    mybir.ActivationFunctionType.Sign,
                    bias=th[b][:, ib:ib + 1], scale=-1.0,
                    accum_out=acc_a[b][:, ib:ib + 1])
                sd = scrd.tile([128, 1024], F16, tag="sd", name=f"sd_{b}_{ib}")
                nc.vector.tensor_scalar(
                    sd[:, :wd], pd[:, :wd],
                    th[b][:, ib:ib + 1], None,
                    op0=mybir.AluOpType.is_lt,
                    op1=mybir.AluOpType.add,
                    accum_out=acc_d[b][:, ib:ib + 1])

            def roundtrip(b):
                # counts: fp32 -> exact fp16 ints -> DRAM -> featT rows
                a16 = accp.tile([128, C], F16, tag="a16", name=f"a16_{b}")
                nc.scalar.copy(a16[:], acc_a[b][:])
                d16 = accp.tile([128, C], F16, tag="d16", name=f"d16_{b}")
                nc.scalar.copy(d16[:], acc_d[b][:])
                nc.gpsimd.dma_start(sd_a[b][:, :], a16[:])
                nc.gpsimd.dma_start(sd_d[b][:, :], d16[:])
                row_a = sd_a[b][:, :].rearrange("p c -> (p c)") \
                    .rearrange("(a n) -> a n", a=1)
                row_d = sd_d[b][:, :].rearrange("p c -> (p c)") \
                    .rearrange("(a n) -> a n", a=1)
                # featT rows per group: [pl, pl, S, S, C, C, 1, 1]
                for grp in (0, 32, 64, 96):
                    nc.gpsimd.dma_start(ft[b][grp + 2:grp + 3, :], row_a)
                    nc.gpsimd.dma_start(ft[b][grp + 3:grp + 4, :], row_a)
                    nc.gpsimd.dma_start(ft[b][grp + 4:grp + 5, :], row_d)
                    nc.gpsimd.dma_start(ft[b][grp + 5:grp + 6, :], row_d)

            def proj_pair(b, ib2):
                osb = outp.tile([128, 768], F32, tag="osb",
                                name=f"osb_{b}_{ib2}")
                pool = pap if (ib2 // 2) % 2 == 0 else pdp
                tag = "pa" if pool is pap else "pd"
                po = pool.tile([128, 1024], F32, tag=tag,
                               name=f"po_{b}_{ib2}")
                for s_ in range(2):
                    ib = ib2 + s_
                    grp = 32 * (ib % 4)
                    nc.tensor.matmul(
                        po[:, s_ * 512:s_ * 512 + 384],
                        ft[b][grp:grp + 8, ib * 128:(ib + 1) * 128],
                        rhsW[grp:grp + 8, :], start=True, stop=True,
                        tile_position=(grp, 0))
                pov = po[:].rearrange("p (s k) -> p s k", s=2)[:, :, :384]
                if (ib2 // 2) % 2 == 0:
                    nc.vector.tensor_copy(
                        osb[:].rearrange("p (s k) -> p s k", s=2), pov)
                else:
                    nc.scalar.copy(
                        osb[:].rearrange("p (s k) -> p s k", s=2), pov)
                dst = pj_d.ap()[b, ib2 * 128:(ib2 + 2) * 128, :] \
                    .rearrange("(s p) k -> p s k", p=128)
                deng = (nc.sync, nc.gpsimd)[(ib2 // 2) % 2]
                deng.dma_start(
                    dst, osb[:].rearrange("p (s k) -> p s k", s=2))

            for b in range(BPC):
                npairs = 0
                for ib in range(NB):
                    counts_block(b, ib)
                    if b > 0 and ib >= 3 and ib % 2 == 1:
                        proj_pair(b - 1, 2 * npairs)
                        npairs += 1
                roundtrip(b)
                if b > 0:
                    while npairs < NB // 2:
                        proj_pair(b - 1, 2 * npairs)
                        npairs += 1
            for ib2 in range(0, NB, 2):
                proj_pair(BPC - 1, ib2)
    nc.compile()
    return nc


def prep_core_inputs(action_mask, keepout, probe, locs, W, J, wa):
    """Host-side prep for one core's batches. Returns in_map dict."""
    BPC, N, _ = locs.shape
    C = N // 128

    placed = (~action_mask) & ~(keepout | probe)          # [BPC, N] bool
    placed_f = placed.astype(np.float32)
    x = locs.astype(np.float32)
    sq = (x ** 2).sum(-1)                                 # [BPC, N]
    thresh = (R2 - sq).astype(np.float32)

    # p-major i-block layout: lhsT col m of block ib <- i = m*C + ib
    m = np.arange(128)
    src = (m[None, :] * C + np.arange(C)[:, None]).reshape(-1)  # pos ib*128+m

    xi = np.zeros((BPC, 8, N), np.float16)
    rhs3 = np.zeros((BPC, 8, J), np.float16)
    featT = np.zeros((BPC, 8, N), np.float16)
    th_pm = np.zeros((BPC, 128, C), np.float32)

    for b in range(BPC):
        x0h, x0l = _split16(x[b, :, 0])
        x1h, x1l = _split16(x[b, :, 1])

        idx = np.nonzero(placed[b])[0]
        np_ = len(idx)
        assert np_ <= J, f"placed count {np_} exceeds J={J}"
        j0h, j0l = _split16(-2.0 * x[b, idx, 0])
        j1h, j1l = _split16(-2.0 * x[b, idx, 1])
        sqh, sql = _split16(sq[b, idx])

        # lhsT rows: [xh0, xh0, xl0, xh1, xh1, xl1, 1, 1]
        xi[b, 0, :] = x0h[src]
        xi[b, 1, :] = x0h[src]
        xi[b, 2, :] = x0l[src]
        xi[b, 3, :] = x1h[src]
        xi[b, 4, :] = x1h[src]
        xi[b, 5, :] = x1l[src]
        xi[b, 6, :] = 1.0
        xi[b, 7, :] = 1.0
        # rhs rows pair to give xh*(-2xh) + xh*(-2xl) + xl*(-2xh)
        # per coord, plus 1*sqh + 1*sql (pads: sq = BIG).
        rhs3[b, 6, :] = BIG
        rhs3[b, 0, :np_] = j0h
        rhs3[b, 1, :np_] = j0l
        rhs3[b, 2, :np_] = j0h
        rhs3[b, 3, :np_] = j1h
        rhs3[b, 4, :np_] = j1l
        rhs3[b, 5, :np_] = j1h
        rhs3[b, 6, :np_] = sqh
        rhs3[b, 7, :np_] = sql
        # featT rows: [pl, pl, S, S, C, C, 1, 1]; S/C filled on device
        featT[b, 0, :] = placed_f[b]
        featT[b, 1, :] = placed_f[b]
        featT[b, 6, :] = 1.0
        featT[b, 7, :] = 1.0

        th_pm[b] = thresh[b].reshape(128, C)

    W = W.astype(np.float32)
    rhsW = np.zeros((8, 384), np.float16)
    rows = [W[0],                         # placed
            W[1] / (2.0 * SCALE),         # S (ACT sign-sum)
            W[1] / SCALE,                 # C (DVE count)
            (wa / (2.0 * SCALE)) * W[1]]  # ones (Sign affine fix)
    for r, v in enumerate(rows):
        h, lo = _split16(v)
        rhsW[2 * r] = h
        rhsW[2 * r + 1] = lo

    return {"xi": xi, "rhs3": rhs3, "thresh": th_pm, "featT": featT,
            "rhsW": rhsW}


_PROGRAM_CACHE = {}


def kernel(action_mask, keepout, probe, locs, W, _trace=False, _tmpdir=None):
    action_mask = np.asarray(action_mask)
    keepout = np.asarray(keepout)
    probe = np.asarray(probe)
    locs = np.asarray(locs, dtype=np.float32)
    W = np.asarray(W, dtype=np.float32)

    B, N = action_mask.shape
    BPC = B // N_CORES

    placed = (~action_mask) & ~(keepout | probe)
    max_placed = int(placed.sum(1).max())
    J = max(1536, ((max_placed + 63) // 64) * 64)
    wa = _wa_for(J)

    key = (N, BPC, J, wa)
    if key not in _PROGRAM_CACHE:
        _PROGRAM_CACHE[key] = build_program(N, BPC, J, wa)
    nc = _PROGRAM_CACHE[key]

    in_maps = []
    for c in range(N_CORES):
        s = slice(c * BPC, (c + 1) * BPC)
        in_maps.append(prep_core_inputs(
            action_mask[s], keepout[s], probe[s], locs[s], W, J, wa))

    res = bass_utils.run_bass_kernel_spmd(
        nc, in_maps, core_ids=list(range(N_CORES)),
        trace=_trace, tmpdir=_tmpdir)

    proj = np.concatenate([res.results[c]["proj"] for c in range(N_CORES)], 0)
    out = (np.ascontiguousarray(proj[:, :, :128]),
           np.ascontiguousarray(proj[:, :, 128:256]),
           np.ascontiguousarray(proj[:, :, 256:384]))
    if _trace:
        return out, res
    return out


# revision 12
# speedup vs baseline: 1.2016x; 1.1316x over previous
"""Trainium2 Bass kernel for DPPDynamicEmbedding (retrieval_knn).

Reference computation (per batch b, N=4096 points in [0,1]^2):
  placed    = (~action_mask) & ~(keepout | probe)                  [N] bool
  d2[i,j]   = |x_i|^2 + |x_j|^2 - 2 x_i.x_j                        [N,N]
  density_i = |{j : placed_j and d2[i,j] < R^2}| / 20              [N]
  proj      = [placed, density] @ W                                [N, 384]
  out       = split(proj, 3) -> (glimpse_key, glimpse_val, logit_key)

Strategy: data-parallel, 2 batches per core on 8 cores.  Per i-block of
128 points (i = p*C + ib, p = SBUF partition, C = N/128):

  - PE computes psum[i, j] = -2 x_i.x_j + masked_sq_j with K=8 fp16
    matmuls.  fp16 hi/lo splitting (x = xh + xl, sq = sqh + sql) keeps
    d2 accurate to ~1e-6 while running single-pass (fp32 matmuls are
    2-pass on TRN2 and much slower).  The j-domain is compacted on the
    host to only placed points, padded to J; pad entries get sq = 32768
    so they can never be inside the radius.
  - K=8 fits a 32-row PE group, so the per-block matmuls cycle through
    array row groups 0/32/64/96 via tile_position; weights and moving
    operands are replicated at partition offsets 0/32/64/96 so the four
    chunk matmuls (and the rotating projection matmul) overlap in the
    array instead of serializing on the ~(219+N)cyc isolated-MM latency.
  - The comparison d2 < R^2  <=>  psum < thresh_i (thresh = R^2 - sq_i)
    is one fused compare+count op per engine per i-block, reading a
    2-bank PSUM tile:
      ACT: Sign(thresh - psum), accum_out S  => count = (S + Wa)/2
      DVE: tensor_scalar is_lt, accum_out C  => count = C
  - Counts (fp32, converted to exact fp16 integers) take a tiny DRAM
    round-trip to become rows [1, N] (p-major flatten == i order) of
    feat^T, written into all four row-group replicas.
  - Projection is one K=8 fp16 matmul per 128 points: lhsT rows =
    [placed, placed, S, S, C, C, 1, 1], rhs rows = hi/lo splits of
    [W0, W1/40, W1/20, (Wa/40) W1] -- count normalization and the
    Sign->count affine fix ride the matmul; the hi/lo W split keeps
    fp32-level accuracy.  PSUM -> SBUF copy (ACT/DVE split; DMA cannot
    read PSUM) then one DMA per two i-blocks into a [N, 384] output.
"""

import numpy as np

import concourse.bass as bass
import concourse.mybir as mybir
import concourse.tile as tile
from concourse import bacc, bass_utils

R2 = 0.16
SCALE = 20.0
BIG = 32768.0          # pad sentinel; must be fp16-exact and >> R2
N_CORES = 8

F32 = mybir.dt.float32
F16 = mybir.dt.float16


def _split16(v):
    """Split fp32 array into (hi, lo) fp16 pair with v ~= hi + lo."""
    hi = v.astype(np.float16)
    lo = (v - hi.astype(np.float32)).astype(np.float16)
    return hi, lo


def _wa_for(J):
    """ACT-side j-width (DVE gets J - wa)."""
    return int(max(512, min(1024, round(J * 0.483 / 64) * 64)))


def _subchunks(w, lim=512):
    out = []
    off = 0
    while off < w:
        out.append((off, min(lim, w - off)))
        off += lim
    return out


def build_program(N, BPC, J, wa, copy_split=4):
    """N points, BPC batches per core, J padded j-count, wa = ACT width."""
    C = N // 128
    NB = N // 128
    wd = J - wa
    # chunk list: (psum kind, psum offset, rhs offset, width, row group)
    chunks = []
    g = 0
    for off, w in _subchunks(wa):
        chunks.append(("a", off, off, w, 32 * (g % 4)))
        g += 1
    for off, w in _subchunks(wd):
        chunks.append(("d", off, wa + off, w, 32 * (g % 4)))
        g += 1

    nc = bacc.Bacc("TRN2", target_bir_lowering=False, debug=False,
                   num_devices=N_CORES)

    xi_d = nc.dram_tensor("xi", [BPC, 8, N], F16, kind="ExternalInput")
    rhs3_d = nc.dram_tensor("rhs3", [BPC, 8, J], F16, kind="ExternalInput")
    th_d = nc.dram_tensor("thresh", [BPC, 128, C], F32, kind="ExternalInput")
    ft_d = nc.dram_tensor("featT", [BPC, 8, N], F16, kind="ExternalInput")
    rw_d = nc.dram_tensor("rhsW", [8, 384], F16, kind="ExternalInput")
    pj_d = nc.dram_tensor("proj", [BPC, N, 384], F32, kind="ExternalOutput")

    with tile.TileContext(nc) as tc:
        with (
            tc.tile_pool(name="const", bufs=BPC) as cpool,
            tc.tile_pool(name="accp", bufs=BPC) as accp,
            tc.tile_pool(name="pa", bufs=2, space="PSUM") as pap,
            tc.tile_pool(name="pdv", bufs=2, space="PSUM") as pdp,
            tc.tile_pool(name="scr_a", bufs=2) as scra,
            tc.tile_pool(name="scr_d", bufs=2) as scrd,
            tc.tile_pool(name="outsb", bufs=4) as outp,
            tc.tile_pool(name="dram", bufs=BPC, space="DRAM") as dram,
            tc.tile_pool(name="w", bufs=1) as wpool,
        ):
            def load_repl(pool, tag, name, dram_ap, rows, cols, eng):
                t = pool.tile([128, cols], F16, tag=tag, name=name)
                eng.dma_start(t[:rows, :], dram_ap)
                for grp in (32, 64, 96):
                    eng.dma_start(t[grp:grp + rows, :], t[:rows, :])
                return t

            xi, rhs3, th, ft, acc_a, acc_d, sd_a, sd_d = \
                [], [], [], [], [], [], [], []
            for b in range(BPC):
                eng = nc.sync if b == 0 else nc.gpsimd
                xi.append(load_repl(cpool, "xi", f"xi{b}", xi_d.ap()[b],
                                    8, N, eng))
                rhs3.append(load_repl(cpool, "rhs3", f"rhs3{b}",
                                      rhs3_d.ap()[b], 8, J, eng))
                t = cpool.tile([128, C], F32, tag="th", name=f"th{b}")
                eng.dma_start(t[:], th_d.ap()[b])
                th.append(t)
            rhsW = load_repl(wpool, "rw", "rhsW", rw_d.ap(), 8, 384,
                             nc.gpsimd)
            for b in range(BPC):
                ft.append(load_repl(cpool, "ft", f"ft{b}", ft_d.ap()[b],
                                    8, N, nc.gpsimd))
            for b in range(BPC):
                acc_a.append(accp.tile([128, C], F32, tag="aa", name=f"aa{b}"))
                acc_d.append(accp.tile([128, C], F32, tag="ad", name=f"ad{b}"))
                sd_a.append(dram.tile([4, 128, C], F16, tag="sa",
                                      name=f"sda{b}"))

            # ---- fused phases: counts(b) interleaved with proj(b-1) ----
            def counts_block(b, ib):
                isl = slice(ib * 128, (ib + 1) * 128)
                pa = pap.tile([128, 1024], F32, tag="pa", name=f"pa_{b}_{ib}")
                pd = pdp.tile([128, 1024], F32, tag="pd", name=f"pd_{b}_{ib}")
                for kind, po_, ro, w, grp in chunks:
                    dst = pa if kind == "a" else pd
                    nc.tensor.matmul(
                        dst[:, po_:po_ + w],
                        xi[b][grp:grp + 8, isl],
                        rhs3[b][grp:grp + 8, ro:ro + w],
                        start=True, stop=True,
                        tile_position=(grp, 0))
                sa = scra.tile([128, 1024], F16, tag="sa", name=f"sa_{b}_{ib}")
                nc.scalar.activation(
                    sa[:, :wa], pa[:, :wa],
                    mybir.ActivationFunctionType.Sign,
                    bias=th[b][:, ib:ib + 1], scale=-1.0,
                    accum_out=acc_a[b][:, ib:ib + 1])
                sd = scrd.tile([128, 1024], F16, tag="sd", name=f"sd_{b}_{ib}")
                nc.vector.tensor_scalar(
                    sd[:, :wd], pd[:, :wd],
                    th[b][:, ib:ib + 1], None,
                    op0=mybir.AluOpType.is_lt,
                    op1=mybir.AluOpType.add,
                    accum_out=acc_d[b][:, ib:ib + 1])

            def roundtrip(b):
                # counts: fp32 -> exact fp16 ints -> DRAM -> featT rows
                # scratch rows [S, S, C, C]; p-major flatten == i order.
                a16 = accp.tile([128, C], F16, tag="a16", name=f"a16_{b}")
                nc.scalar.copy(a16[:], acc_a[b][:])
                d16 = accp.tile([128, C], F16, tag="d16", name=f"d16_{b}")
                nc.scalar.copy(d16[:], acc_d[b][:])
                for r, t16 in ((0, a16), (1, a16), (2, d16), (3, d16)):
                    nc.sync.dma_start(sd_a[b][r, :, :], t16[:])
                rows4 = sd_a[b][:, :, :].rearrange("r p c -> r (p c)")
                for grp in (0, 32, 64, 96):
                    nc.sync.dma_start(ft[b][grp + 2:grp + 6, :], rows4)

            def proj_pair(b, ib2):
                osb = outp.tile([128, 768], F32, tag="osb",
                                name=f"osb_{b}_{ib2}")
                pool = pap if (ib2 // 2) % 2 == 0 else pdp
                tag = "pa" if pool is pap else "pd"
                po = pool.tile([128, 1024], F32, tag=tag,
                               name=f"po_{b}_{ib2}")
                for s_ in range(2):
                    ib = ib2 + s_
                    grp = 32 * (ib % 4)
                    nc.tensor.matmul(
                        po[:, s_ * 512:s_ * 512 + 384],
                        ft[b][grp:grp + 8, ib * 128:(ib + 1) * 128],
                        rhsW[grp:grp + 8, :], start=True, stop=True,
                        tile_position=(grp, 0))
                pov = po[:].rearrange("p (s k) -> p s k", s=2)[:, :, :384]
                if (ib2 // 2) % 2 == 0:
                    nc.vector.tensor_copy(
                        osb[:].rearrange("p (s k) -> p s k", s=2), pov)
                else:
                    nc.scalar.copy(
                        osb[:].rearrange("p (s k) -> p s k", s=2), pov)
                dst = pj_d.ap()[b, ib2 * 128:(ib2 + 2) * 128, :] \
                    .rearrange("(s p) k -> p s k", p=128)
                deng = (nc.sync, nc.gpsimd, nc.scalar)[(ib2 // 2) % 3]
                deng.dma_start(
                    dst, osb[:].rearrange("p (s k) -> p s k", s=2))

            for b in range(BPC):
                npairs = 0
                for ib in range(NB):
                    counts_block(b, ib)
                    if b > 0 and ib >= 3 and ib % 2 == 1:
                        proj_pair(b - 1, 2 * npairs)
                        npairs += 1
                roundtrip(b)
                if b > 0:
                    while npairs < NB // 2:
                        proj_pair(b - 1, 2 * npairs)
                        npairs += 1
            for ib2 in range(0, NB, 2):
                proj_pair(BPC - 1, ib2)
    nc.compile()
    return nc


def prep_core_inputs(action_mask, keepout, probe, locs, W, J, wa):
    """Host-side prep for one core's batches. Returns in_map dict."""
    BPC, N, _ = locs.shape
    C = N // 128

    placed = (~action_mask) & ~(keepout | probe)          # [BPC, N] bool
    placed_f = placed.astype(np.float32)
    x = locs.astype(np.float32)
    sq = (x ** 2).sum(-1)                                 # [BPC, N]
    thresh = (R2 - sq).astype(np.float32)

    # p-major i-block layout: lhsT col m of block ib <- i = m*C + ib
    m = np.arange(128)
    src = (m[None, :] * C + np.arange(C)[:, None]).reshape(-1)  # pos ib*128+m

    xi = np.zeros((BPC, 8, N), np.float16)
    rhs3 = np.zeros((BPC, 8, J), np.float16)
    featT = np.zeros((BPC, 8, N), np.float16)
    th_pm = np.zeros((BPC, 128, C), np.float32)

    for b in range(BPC):
        x0h, x0l = _split16(x[b, :, 0])
        x1h, x1l = _split16(x[b, :, 1])

        idx = np.nonzero(placed[b])[0]
        np_ = len(idx)
        assert np_ <= J, f"placed count {np_} exceeds J={J}"
        j0h, j0l = _split16(-2.0 * x[b, idx, 0])
        j1h, j1l = _split16(-2.0 * x[b, idx, 1])
        sqh, sql = _split16(sq[b, idx])

        # lhsT rows: [xh0, xh0, xl0, xh1, xh1, xl1, 1, 1]
        xi[b, 0, :] = x0h[src]
        xi[b, 1, :] = x0h[src]
        xi[b, 2, :] = x0l[src]
        xi[b, 3, :] = x1h[src]
        xi[b, 4, :] = x1h[src]
        xi[b, 5, :] = x1l[src]
        xi[b, 6, :] = 1.0
        xi[b, 7, :] = 1.0
        # rhs rows pair to give xh*(-2xh) + xh*(-2xl) + xl*(-2xh)
        # per coord, plus 1*sqh + 1*sql (pads: sq = BIG).
        rhs3[b, 6, :] = BIG
        rhs3[b, 0, :np_] = j0h
        rhs3[b, 1, :np_] = j0l
        rhs3[b, 2, :np_] = j0h
        rhs3[b, 3, :np_] = j1h
        rhs3[b, 4, :np_] = j1l
        rhs3[b, 5, :np_] = j1h
        rhs3[b, 6, :np_] = sqh
        rhs3[b, 7, :np_] = sql
        # featT rows: [pl, pl, S, S, C, C, 1, 1]; S/C filled on device
        featT[b, 0, :] = placed_f[b]
        featT[b, 1, :] = placed_f[b]
        featT[b, 6, :] = 1.0
        featT[b, 7, :] = 1.0

        th_pm[b] = thresh[b].reshape(128, C)

    W = W.astype(np.float32)
    rhsW = np.zeros((8, 384), np.float16)
    rows = [W[0],                         # placed
            W[1] / (2.0 * SCALE),         # S (ACT sign-sum)
            W[1] / SCALE,                 # C (DVE count)
            (wa / (2.0 * SCALE)) * W[1]]  # ones (Sign affine fix)
    for r, v in enumerate(rows):
        h, lo = _split16(v)
        rhsW[2 * r] = h
        rhsW[2 * r + 1] = lo

    return {"xi": xi, "rhs3": rhs3, "thresh": th_pm, "featT": featT,
            "rhsW": rhsW}


_PROGRAM_CACHE = {}


def kernel(action_mask, keepout, probe, locs, W, _trace=False, _tmpdir=None):
    action_mask = np.asarray(action_mask)
    keepout = np.asarray(keepout)
    probe = np.asarray(probe)
    locs = np.asarray(locs, dtype=np.float32)
    W = np.asarray(W, dtype=np.float32)

    B, N = action_mask.shape
    BPC = B // N_CORES

    placed = (~action_mask) & ~(keepout | probe)
    max_placed = int(placed.sum(1).max())
    J = max(1536, ((max_placed + 63) // 64) * 64)
    wa = _wa_for(J)

    key = (N, BPC, J, wa)
    if key not in _PROGRAM_CACHE:
        _PROGRAM_CACHE[key] = build_program(N, BPC, J, wa)
    nc = _PROGRAM_CACHE[key]

    in_maps = []
    for c in range(N_CORES):
        s = slice(c * BPC, (c + 1) * BPC)
        in_maps.append(prep_core_inputs(
            action_mask[s], keepout[s], probe[s], locs[s], W, J, wa))

    res = bass_utils.run_bass_kernel_spmd(
        nc, in_maps, core_ids=list(range(N_CORES)),
        trace=_trace, tmpdir=_tmpdir)

    proj = np.concatenate([res.results[c]["proj"] for c in range(N_CORES)], 0)
    out = (np.ascontiguousarray(proj[:, :, :128]),
           np.ascontiguousarray(proj[:, :, 128:256]),
           np.ascontiguousarray(proj[:, :, 256:384]))
    if _trace:
        return out, res
    return out


# revision 13
# speedup vs baseline: 1.2108x; 1.0076x over previous
"""Trainium2 Bass kernel for DPPDynamicEmbedding (retrieval_knn).

Reference computation (per batch b, N=4096 points in [0,1]^2):
  placed    = (~action_mask) & ~(keepout | probe)                  [N] bool
  d2[i,j]   = |x_i|^2 + |x_j|^2 - 2 x_i.x_j                        [N,N]
  density_i = |{j : placed_j and d2[i,j] < R^2}| / 20              [N]
  proj      = [placed, density] @ W                                [N, 384]
  out       = split(proj, 3) -> (glimpse_key, glimpse_val, logit_key)

Strategy: data-parallel, 2 batches per core on 8 cores.  Per i-block of
128 points (i = p*C + ib, p = SBUF partition, C = N/128):

  - PE computes psum[i, j] = -2 x_i.x_j + masked_sq_j with K=8 fp16
    matmuls.  fp16 hi/lo splitting (x = xh + xl, sq = sqh + sql) keeps
    d2 accurate to ~1e-6 while running single-pass (fp32 matmuls are
    2-pass on TRN2 and much slower).  The j-domain is compacted on the
    host to only placed points, padded to J; pad entries get sq = 32768
    so they can never be inside the radius.
  - K=8 fits a 32-row PE group, so the per-block matmuls cycle through
    array row groups 0/32/64/96 via tile_position; weights and moving
    operands are replicated at partition offsets 0/32/64/96 so the four
    chunk matmuls (and the rotating projection matmul) overlap in the
    array instead of serializing on the ~(219+N)cyc isolated-MM latency.
  - The comparison d2 < R^2  <=>  psum < thresh_i (thresh = R^2 - sq_i)
    is one fused compare+count op per engine per i-block, reading a
    2-bank PSUM tile:
      ACT: Sign(thresh - psum), accum_out S  => count = (S + Wa)/2
      DVE: tensor_scalar is_lt, accum_out C  => count = C
  - Counts (fp32, converted to exact fp16 integers) take a tiny DRAM
    round-trip to become rows [1, N] (p-major flatten == i order) of
    feat^T, written into all four row-group replicas.
  - Projection is one K=8 fp16 matmul per 128 points: lhsT rows =
    [placed, placed, S, S, C, C, 1, 1], rhs rows = hi/lo splits of
    [W0, W1/40, W1/20, (Wa/40) W1] -- count normalization and the
    Sign->count affine fix ride the matmul; the hi/lo W split keeps
    fp32-level accuracy.  PSUM -> SBUF copy (ACT/DVE split; DMA cannot
    read PSUM) then one DMA per two i-blocks into a [N, 384] output.
"""

import numpy as np

import concourse.bass as bass
import concourse.mybir as mybir
import concourse.tile as tile
from concourse import bacc, bass_utils

R2 = 0.16
SCALE = 20.0
BIG = 32768.0          # pad sentinel; must be fp16-exact and >> R2
N_CORES = 8

F32 = mybir.dt.float32
F16 = mybir.dt.float16


def _split16(v):
    """Split fp32 array into (hi, lo) fp16 pair with v ~= hi + lo."""
    hi = v.astype(np.float16)
    lo = (v - hi.astype(np.float32)).astype(np.float16)
    return hi, lo


def _wa_for(J):
    """ACT-side j-width (DVE gets J - wa)."""
    return int(max(512, min(1024, round(J * 0.483 / 64) * 64)))


def _subchunks(w, lim=512):
    out = []
    off = 0
    while off < w:
        out.append((off, min(lim, w - off)))
        off += lim
    return out


def build_program(N, BPC, J, wa, copy_split=4):
    """N points, BPC batches per core, J padded j-count, wa = ACT width."""
    C = N // 128
    NB = N // 128
    wd = J - wa
    # chunk list: (psum kind, psum offset, rhs offset, width, row group)
    chunks = []
    g = 0
    for off, w in _subchunks(wa):
        chunks.append(("a", off, off, w, 32 * (g % 4)))
        g += 1
    for off, w in _subchunks(wd):
        chunks.append(("d", off, wa + off, w, 32 * (g % 4)))
        g += 1

    nc = bacc.Bacc("TRN2", target_bir_lowering=False, debug=False,
                   num_devices=N_CORES)

    xi_d = nc.dram_tensor("xi", [BPC, 8, N], F16, kind="ExternalInput")
    rhs3_d = nc.dram_tensor("rhs3", [BPC, 8, J], F16, kind="ExternalInput")
    th_d = nc.dram_tensor("thresh", [BPC, 128, C], F32, kind="ExternalInput")
    ft_d = nc.dram_tensor("featT", [BPC, 8, N], F16, kind="ExternalInput")
    rw_d = nc.dram_tensor("rhsW", [8, 384], F16, kind="ExternalInput")
    pj_d = nc.dram_tensor("proj", [BPC, N, 384], F32, kind="ExternalOutput")

    with tile.TileContext(nc) as tc:
        with (
            tc.tile_pool(name="const", bufs=BPC) as cpool,
            tc.tile_pool(name="accp", bufs=BPC) as accp,
            tc.tile_pool(name="pa", bufs=2, space="PSUM") as pap,
            tc.tile_pool(name="pdv", bufs=2, space="PSUM") as pdp,
            tc.tile_pool(name="scr_a", bufs=2) as scra,
            tc.tile_pool(name="scr_d", bufs=2) as scrd,
            tc.tile_pool(name="outsb", bufs=4) as outp,
            tc.tile_pool(name="dram", bufs=BPC, space="DRAM") as dram,
            tc.tile_pool(name="w", bufs=1) as wpool,
        ):
            def load_repl(pool, tag, name, dram_ap, rows, cols, eng):
                t = pool.tile([128, cols], F16, tag=tag, name=name)
                eng.dma_start(t[:rows, :], dram_ap)
                engs = ((nc.sync, nc.gpsimd, nc.scalar) if eng is nc.sync
                        else (nc.gpsimd,) * 3)
                for e, grp in zip(engs, (32, 64, 96)):
                    e.dma_start(t[grp:grp + rows, :], t[:rows, :])
                return t

            xi, rhs3, th, ft, acc_a, acc_d, sd_a, sd_d = \
                [], [], [], [], [], [], [], []
            for b in range(BPC):
                eng = nc.sync if b == 0 else nc.gpsimd
                xi.append(load_repl(cpool, "xi", f"xi{b}", xi_d.ap()[b],
                                    8, N, eng))
                rhs3.append(load_repl(cpool, "rhs3", f"rhs3{b}",
                                      rhs3_d.ap()[b], 8, J, eng))
                t = cpool.tile([128, C], F32, tag="th", name=f"th{b}")
                eng.dma_start(t[:], th_d.ap()[b])
                th.append(t)
            rhsW = load_repl(wpool, "rw", "rhsW", rw_d.ap(), 8, 384,
                             nc.gpsimd)
            for b in range(BPC):
                ft.append(load_repl(cpool, "ft", f"ft{b}", ft_d.ap()[b],
                                    8, N, nc.gpsimd))
            for b in range(BPC):
                acc_a.append(accp.tile([128, C], F32, tag="aa", name=f"aa{b}"))
                acc_d.append(accp.tile([128, C], F32, tag="ad", name=f"ad{b}"))
                sd_a.append(dram.tile([4, 128, C], F16, tag="sa",
                                      name=f"sda{b}"))

            # ---- fused phases: counts(b) interleaved with proj(b-1) ----
            def counts_block(b, ib):
                isl = slice(ib * 128, (ib + 1) * 128)
                pa = pap.tile([128, 1024], F32, tag="pa", name=f"pa_{b}_{ib}")
                pd = pdp.tile([128, 1024], F32, tag="pd", name=f"pd_{b}_{ib}")
                for kind, po_, ro, w, grp in chunks:
                    dst = pa if kind == "a" else pd
                    nc.tensor.matmul(
                        dst[:, po_:po_ + w],
                        xi[b][grp:grp + 8, isl],
                        rhs3[b][grp:grp + 8, ro:ro + w],
                        start=True, stop=True,
                        tile_position=(grp, 0))
                sa = scra.tile([128, 1024], F16, tag="sa", name=f"sa_{b}_{ib}")
                nc.scalar.activation(
                    sa[:, :wa], pa[:, :wa],
                    mybir.ActivationFunctionType.Sign,
                    bias=th[b][:, ib:ib + 1], scale=-1.0,
                    accum_out=acc_a[b][:, ib:ib + 1])
                sd = scrd.tile([128, 1024], F16, tag="sd", name=f"sd_{b}_{ib}")
                nc.vector.tensor_scalar(
                    sd[:, :wd], pd[:, :wd],
                    th[b][:, ib:ib + 1], None,
                    op0=mybir.AluOpType.is_lt,
                    op1=mybir.AluOpType.add,
                    accum_out=acc_d[b][:, ib:ib + 1])

            def roundtrip(b):
                # counts: fp32 -> exact fp16 ints -> DRAM -> featT rows
                # scratch rows [S, S, C, C]; p-major flatten == i order.
                a16 = accp.tile([128, C], F16, tag="a16", name=f"a16_{b}")
                nc.scalar.copy(a16[:], acc_a[b][:])
                d16 = accp.tile([128, C], F16, tag="d16", name=f"d16_{b}")
                nc.scalar.copy(d16[:], acc_d[b][:])
                for r, t16 in ((0, a16), (1, a16), (2, d16), (3, d16)):
                    nc.sync.dma_start(sd_a[b][r, :, :], t16[:])
                rows4 = sd_a[b][:, :, :].rearrange("r p c -> r (p c)")
                for grp in (0, 32, 64, 96):
                    nc.sync.dma_start(ft[b][grp + 2:grp + 6, :], rows4)

            def proj_pair(b, ib2):
                osb = outp.tile([128, 768], F32, tag="osb",
                                name=f"osb_{b}_{ib2}")
                pool = pap if (ib2 // 2) % 2 == 0 else pdp
                tag = "pa" if pool is pap else "pd"
                po = pool.tile([128, 1024], F32, tag=tag,
                               name=f"po_{b}_{ib2}")
                for s_ in range(2):
                    ib = ib2 + s_
                    grp = 32 * (ib % 4)
                    nc.tensor.matmul(
                        po[:, s_ * 512:s_ * 512 + 384],
                        ft[b][grp:grp + 8, ib * 128:(ib + 1) * 128],
                        rhsW[grp:grp + 8, :], start=True, stop=True,
                        tile_position=(grp, 0))
                pov = po[:].rearrange("p (s k) -> p s k", s=2)[:, :, :384]
                if (ib2 // 2) % 2 == 0:
                    nc.vector.tensor_copy(
                        osb[:].rearrange("p (s k) -> p s k", s=2), pov)
                else:
                    nc.scalar.copy(
                        osb[:].rearrange("p (s k) -> p s k", s=2), pov)
                dst = pj_d.ap()[b, ib2 * 128:(ib2 + 2) * 128, :] \
                    .rearrange("(s p) k -> p s k", p=128)
                deng = (nc.sync, nc.gpsimd, nc.scalar)[(ib2 // 2) % 3]
                deng.dma_start(
                    dst, osb[:].rearrange("p (s k) -> p s k", s=2))

            for b in range(BPC):
                npairs = 0
                for ib in range(NB):
                    counts_block(b, ib)
                    if b > 0 and ib >= 3 and ib % 2 == 1:
                        proj_pair(b - 1, 2 * npairs)
                        npairs += 1
                        if ib >= NB - 4 and npairs < NB // 2:
                            proj_pair(b - 1, 2 * npairs)
                            npairs += 1
                roundtrip(b)
                if b > 0:
                    while npairs < NB // 2:
                        proj_pair(b - 1, 2 * npairs)
                        npairs += 1
            for ib2 in range(0, NB, 2):
                proj_pair(BPC - 1, ib2)
    nc.compile()
    return nc


def prep_core_inputs(action_mask, keepout, probe, locs, W, J, wa):
    """Host-side prep for one core's batches. Returns in_map dict."""
    BPC, N, _ = locs.shape
    C = N // 128

    placed = (~action_mask) & ~(keepout | probe)          # [BPC, N] bool
    placed_f = placed.astype(np.float32)
    x = locs.astype(np.float32)
    sq = (x ** 2).sum(-1)                                 # [BPC, N]
    thresh = (R2 - sq).astype(np.float32)

    # p-major i-block layout: lhsT col m of block ib <- i = m*C + ib
    m = np.arange(128)
    src = (m[None, :] * C + np.arange(C)[:, None]).reshape(-1)  # pos ib*128+m

    xi = np.zeros((BPC, 8, N), np.float16)
    rhs3 = np.zeros((BPC, 8, J), np.float16)
    featT = np.zeros((BPC, 8, N), np.float16)
    th_pm = np.zeros((BPC, 128, C), np.float32)

    for b in range(BPC):
        x0h, x0l = _split16(x[b, :, 0])
        x1h, x1l = _split16(x[b, :, 1])

        idx = np.nonzero(placed[b])[0]
        np_ = len(idx)
        assert np_ <= J, f"placed count {np_} exceeds J={J}"
        j0h, j0l = _split16(-2.0 * x[b, idx, 0])
        j1h, j1l = _split16(-2.0 * x[b, idx, 1])
        sqh, sql = _split16(sq[b, idx])

        # lhsT rows: [xh0, xh0, xl0, xh1, xh1, xl1, 1, 1]
        xi[b, 0, :] = x0h[src]
        xi[b, 1, :] = x0h[src]
        xi[b, 2, :] = x0l[src]
        xi[b, 3, :] = x1h[src]
        xi[b, 4, :] = x1h[src]
        xi[b, 5, :] = x1l[src]
        xi[b, 6, :] = 1.0
        xi[b, 7, :] = 1.0
        # rhs rows pair to give xh*(-2xh) + xh*(-2xl) + xl*(-2xh)
        # per coord, plus 1*sqh + 1*sql (pads: sq = BIG).
        rhs3[b, 6, :] = BIG
        rhs3[b, 0, :np_] = j0h
        rhs3[b, 1, :np_] = j0l
        rhs3[b, 2, :np_] = j0h
        rhs3[b, 3, :np_] = j1h
        rhs3[b, 4, :np_] = j1l
        rhs3[b, 5, :np_] = j1h
        rhs3[b, 6, :np_] = sqh
        rhs3[b, 7, :np_] = sql
        # featT rows: [pl, pl, S, S, C, C, 1, 1]; S/C filled on device
        featT[b, 0, :] = placed_f[b]
        featT[b, 1, :] = placed_f[b]
        featT[b, 6, :] = 1.0
        featT[b, 7, :] = 1.0

        th_pm[b] = thresh[b].reshape(128, C)

    W = W.astype(np.float32)
    rhsW = np.zeros((8, 384), np.float16)
    rows = [W[0],                         # placed
            W[1] / (2.0 * SCALE),         # S (ACT sign-sum)
            W[1] / SCALE,                 # C (DVE count)
            (wa / (2.0 * SCALE)) * W[1]]  # ones (Sign affine fix)
    for r, v in enumerate(rows):
        h, lo = _split16(v)
        rhsW[2 * r] = h
        rhsW[2 * r + 1] = lo

    return {"xi": xi, "rhs3": rhs3, "thresh": th_pm, "featT": featT,
            "rhsW": rhsW}


_PROGRAM_CACHE = {}


def kernel(action_mask, keepout, probe, locs, W, _trace=False, _tmpdir=None):
    action_mask = np.asarray(action_mask)
    keepout = np.asarray(keepout)
    probe = np.asarray(probe)
    locs = np.asarray(locs, dtype=np.float32)
    W = np.asarray(W, dtype=np.float32)

    B, N = action_mask.shape
    BPC = B // N_CORES

    placed = (~action_mask) & ~(keepout | probe)
    max_placed = int(placed.sum(1).max())
    J = max(1536, ((max_placed + 63) // 64) * 64)
    wa = _wa_for(J)

    key = (N, BPC, J, wa)
    if key not in _PROGRAM_CACHE:
        _PROGRAM_CACHE[key] = build_program(N, BPC, J, wa)
    nc = _PROGRAM_CACHE[key]

    in_maps = []
    for c in range(N_CORES):
        s = slice(c * BPC, (c + 1) * BPC)
        in_maps.append(prep_core_inputs(
            action_mask[s], keepout[s], probe[s], locs[s], W, J, wa))

    res = bass_utils.run_bass_kernel_spmd(
        nc, in_maps, core_ids=list(range(N_CORES)),
        trace=_trace, tmpdir=_tmpdir)

    proj = np.concatenate([res.results[c]["proj"] for c in range(N_CORES)], 0)
    out = (np.ascontiguousarray(proj[:, :, :128]),
           np.ascontiguousarray(proj[:, :, 128:256]),
           np.ascontiguousarray(proj[:, :, 256:384]))
    if _trace:
        return out, res
    return out
